# revision 11
# baseline (speedup 1.0000x reference)
"""Trainium2 Bass kernel for nn_DilatedGCNLayer (masked per-node quantile
aggregation GCN layer).

Contract: kernel(**inputs) takes FULL inputs (x [1024,128] f32, A [1024,1024]
int32, W [128,128] f32, b [128] f32) and returns the FULL [1024,128] f32
output. Internally shards nodes across 8 NeuronCores (128 rows of x/A per
core), replicates x/W/b, runs one SPMD Bass program, and concatenates the
per-core output slabs.

Per-core algorithm (exact, no approximation):
  1. keep/rank phase on [nodes, N]: mask=A!=0, neighbor ranks via prefix
     scan, dilated removal of every skip-th neighbor, keep matrix, n_keep,
     quantile slot indices k_q = max(ceil(q*n_keep)-1, 0).
  2. compact neighbor lists: key = keep*(2048-j); repeated vector.max (8
     maxima/round) + max_index + match_replace extracts each node's kept
     j-indices in ascending-j order; pad slots point at a +FLT_MAX sentinel
     column of xT.
  3. gpsimd.ap_gather pulls xT[:, J] into a slab [128 d, nodes*S].
  4. per-node-segment sort along the free axis with a Batcher odd-even
     merge network (ascending comparators only, so the +FLT_MAX pad tail is
     stable); min/max on VectorE, left-copy on ScalarE, in place.
  5. second ap_gather picks the 3 quantile slots per node (zero sentinel
     for isolated nodes), average, mix with x, single PE matmul with
     0.5*W^T, add bias + residual, transpose back, store.
"""

import numpy as np

# module hyperparameters of the reference nn.Module
LAYER_IDX = 2
T_THRESH = 5
K_DIV = 2
EXP_DILATION = False
QUANTILES = (0.25, 0.5, 0.75)
SELF_RATIO = 0.5
USE_SELF = True
RESIDUAL = True

N_CORES = 8
BIG = float(np.finfo(np.float32).max)
MAGIC = 12582912.0  # 1.5 * 2**23: (v + MAGIC) - MAGIC == round-to-nearest-int


# ---------------------------------------------------------------- sort net
def _batcher_stages(n):
    stages = []
    p = 1
    while p < n:
        k = p
        while k >= 1:
            pairs = []
            j = k % p
            while j + k < n:
                for i in range(0, min(k, n - j - k)):
                    if (i + j) // (p * 2) == (i + j + k) // (p * 2):
                        pairs.append((i + j, i + j + k))
                j += 2 * k
            if pairs:
                stages.append(pairs)
            k //= 2
        p *= 2
    return stages


def _prune_stages(stages, n, keep_out):
    live = np.zeros(n, dtype=bool)
    live[:keep_out] = True
    out = []
    for pairs in reversed(stages):
        kept = []
        for lo, hi in pairs:
            if live[lo] or live[hi]:
                kept.append((lo, hi))
                live[lo] = True
                live[hi] = True
        if kept:
            out.append(kept)
    out.reverse()
    return out


def _pack_families(lefts):
    lefts = sorted(lefts)
    runs = []
    s = prev = lefts[0]
    for v in lefts[1:]:
        if v == prev + 1:
            prev = v
            continue
        runs.append((s, prev - s + 1))
        s = prev = v
    runs.append((s, prev - s + 1))
    fams = []
    i = 0
    while i < len(runs):
        s0, l0 = runs[i]
        j = i + 1
        if j < len(runs) and runs[j][1] == l0:
            step = runs[j][0] - s0
            while (j < len(runs) and runs[j][1] == l0
                   and runs[j][0] == s0 + (j - i) * step):
                j += 1
            fams.append((s0, step, j - i, l0))
        else:
            fams.append((s0, 1, 1, l0))
        i = j
    return fams


def _net_for(n, keep_out):
    stages = _batcher_stages(n)
    if keep_out < n:
        stages = _prune_stages(stages, n, keep_out)
    packed = []
    for pairs in stages:
        dist = pairs[0][1] - pairs[0][0]
        assert all(hi - lo == dist for lo, hi in pairs)
        packed.append((dist, _pack_families([lo for lo, _ in pairs])))
    return packed


# ---------------------------------------------------------------- program
_PROGRAMS = {}


def _build_program(N, D, S):
    import concourse.bass as bass
    import concourse.bacc as bacc
    import concourse.mybir as mybir
    from concourse.tile import TileContext
    from concourse.masks import make_identity

    f32 = mybir.dt.float32
    i32 = mybir.dt.int32
    i16 = mybir.dt.int16
    u32 = mybir.dt.uint32
    Op = mybir.AluOpType

    NODES = N // N_CORES
    NB = NODES
    while NB * S > 16384:
        NB //= 2
    import os
    if (NB == NODES and NB % 2 == 0
            and os.environ.get("DGCN_NO_PIPELINE") != "1"):
        # two batches so the gpsimd gather of batch i+1 overlaps the DVE
        # sort of batch i
        NB //= 2
    NBATCH = NODES // NB
    R = S // 8                      # max8 extraction rounds
    SENT_COL = N                    # xT column holding +BIG
    SLABW = NB * S                  # slab real width
    ZCOL = SLABW                    # slab column holding 0.0 (pick sentinel)
    IOTA_BASE = 2 * N               # key = keep * (IOTA_BASE - j) > 0

    nc = bacc.Bacc(None, target_bir_lowering=False)

    a_slab = nc.dram_tensor("a_slab", [NODES, N], i32, kind="ExternalInput")
    x_full = nc.dram_tensor("x_full", [N, D], f32, kind="ExternalInput")
    xt_own = nc.dram_tensor("xt_own", [D, NODES], f32, kind="ExternalInput")
    w_in = nc.dram_tensor("w_in", [D, D], f32, kind="ExternalInput")
    b_in = nc.dram_tensor("b_in", [D, 1], f32, kind="ExternalInput")
    out_t = nc.dram_tensor("out", [NODES, D], f32, kind="ExternalOutput")

    net = _net_for(S, int(np.ceil(0.75 * S)))
    max_fam_pairs = max(oc * rl for _, fams in net for (_, _, oc, rl) in fams)

    with TileContext(nc) as tc:
        with (
            tc.tile_pool(name="main", bufs=1) as pool,
            tc.tile_pool(name="roll", bufs=3) as roll,
            tc.tile_pool(name="batch", bufs=2) as batchp,
            tc.tile_pool(name="psum", bufs=2, space="PSUM") as psum,
            tc.tile_pool(name="dram", bufs=1, space="DRAM") as dpool,
        ):
            # ---------------- identity for PE transposes
            ident = pool.tile([128, 128], f32, tag="ident")
            make_identity(nc, ident[:, :])

            # ---------------- xT: [D, N+2]; col N..N+2 = +BIG sentinel
            xt = pool.tile([D, N + 2], f32, tag="xt")
            nc.vector.memset(xt[:, N:], BIG)
            for c in range(N // 128):
                xc = roll.tile([128, D], f32, tag="xc")
                nc.sync.dma_start(xc[:, :], x_full[c * 128:(c + 1) * 128, :])
                pt = psum.tile([D, 128], f32, tag="ps")
                nc.tensor.transpose(pt[:, :], xc[:, :], ident[:, :])
                nc.scalar.copy(xt[:, c * 128:(c + 1) * 128], pt[:, :])

            # ---------------- keep / rank phase  [NODES, N]
            a_sb = pool.tile([NODES, N], i32, tag="a_sb")
            nc.sync.dma_start(a_sb[:, :], a_slab[:, :])
            maskf = pool.tile([NODES, N], f32, tag="maskf")
            nc.vector.tensor_scalar(maskf[:, :], a_sb[:, :], 0, None,
                                    op0=Op.not_equal)
            zeros_row = pool.tile([NODES, N], f32, tag="zeros_row")
            nc.vector.memset(zeros_row[:, :], 0.0)
            # rank1[i,j] = inclusive prefix sum of mask = (# nbrs <= j)
            rank1 = pool.tile([NODES, N], f32, tag="rank1")
            nc.vector.tensor_tensor_scan(rank1[:, :], maskf[:, :],
                                         zeros_row[:, :], 0.0,
                                         op0=Op.add, op1=Op.max)
            num_nbr = rank1[:, N - 1:N]

            # skip = where(n > T, ceil(n/K), 1)   (K=2)  [NODES,1]
            half = pool.tile([NODES, 1], f32, tag="half")
            nc.vector.tensor_scalar(half[:, :], num_nbr, 1.0, 0.5,
                                    op0=Op.add, op1=Op.mult)  # (n+1)/2
            rnd = pool.tile([NODES, 1], f32, tag="rnd")
            nc.vector.tensor_scalar(rnd[:, :], half[:, :], MAGIC, -MAGIC,
                                    op0=Op.add, op1=Op.add)
            gt = pool.tile([NODES, 1], f32, tag="gt")
            nc.vector.tensor_tensor(gt[:, :], rnd[:, :], half[:, :], Op.is_gt)
            m_t = pool.tile([NODES, 1], f32, tag="m_t")
            nc.vector.tensor_sub(m_t[:, :], rnd[:, :], gt[:, :])  # floor=ceil(n/2)
            sel = pool.tile([NODES, 1], f32, tag="sel")
            nc.vector.tensor_scalar(sel[:, :], num_nbr, float(T_THRESH), None,
                                    op0=Op.is_gt)
            mm1 = pool.tile([NODES, 1], f32, tag="mm1")
            nc.vector.tensor_scalar(mm1[:, :], m_t[:, :], -1.0, None, op0=Op.add)
            skipv = pool.tile([NODES, 1], f32, tag="skipv")
            nc.vector.scalar_tensor_tensor(skipv[:, :], mm1[:, :], 1.0,
                                           sel[:, :], op0=Op.mult, op1=Op.mult)
            nc.vector.tensor_scalar(skipv[:, :], skipv[:, :], 1.0, None,
                                    op0=Op.add)  # 1 + sel*(m-1)
            recip = pool.tile([NODES, 1], f32, tag="recip")
            nc.vector.reciprocal(recip[:, :], skipv[:, :])

            # keep = mask & ((rank1 % skip) != 0)
            qq = pool.tile([NODES, N], f32, tag="qq")
            nc.vector.tensor_scalar(qq[:, :], rank1[:, :], recip[:, :], None,
                                    op0=Op.mult)
            nc.vector.tensor_scalar(qq[:, :], qq[:, :], MAGIC, -MAGIC,
                                    op0=Op.add, op1=Op.add)  # round(rank/skip)
            prod = pool.tile([NODES, N], f32, tag="prod")
            nc.vector.scalar_tensor_tensor(prod[:, :], qq[:, :], skipv[:, :],
                                           rank1[:, :], op0=Op.mult,
                                           op1=Op.subtract)
            is0 = pool.tile([NODES, N], f32, tag="is0")
            nc.vector.tensor_scalar(is0[:, :], prod[:, :], 0.0, None,
                                    op0=Op.is_equal)
            keep = pool.tile([NODES, N], f32, tag="keep")
            nc.vector.tensor_scalar(is0[:, :], is0[:, :], -1.0, 1.0,
                                    op0=Op.mult, op1=Op.add)  # 1-is0
            nc.vector.tensor_mul(keep[:, :], maskf[:, :], is0[:, :])

            csk = pool.tile([NODES, N], f32, tag="csk")
            nc.vector.tensor_tensor_scan(csk[:, :], keep[:, :], zeros_row[:, :],
                                         0.0, op0=Op.add, op1=Op.max)
            n_keep = csk[:, N - 1:N]

            # quantile slot positions P3[:, q] = i*S + max(ceil(q*n)-1, 0)
            # (q-padded to 8, sentinel ZCOL for pads and isolated nodes)
            # batch-local slab base (i % NB) * S, computed exactly:
            # i/NB is dyadic (NB power of two), floor via the magic-round
            # trick, then base = i*S - floor(i/NB)*NB*S.
            iota_is = pool.tile([NODES, 1], i32, tag="iota_is")
            nc.gpsimd.iota(iota_is[:, :], pattern=[[0, 1]], base=0,
                           channel_multiplier=S)
            iota_isf = pool.tile([NODES, 1], f32, tag="iota_isf")
            nc.vector.tensor_copy(iota_isf[:, :], iota_is[:, :])
            if NBATCH > 1:
                iota_i1 = pool.tile([NODES, 1], i32, tag="iota_i1")
                nc.gpsimd.iota(iota_i1[:, :], pattern=[[0, 1]], base=0,
                               channel_multiplier=1)
                fr = pool.tile([NODES, 1], f32, tag="fr")
                nc.vector.tensor_copy(fr[:, :], iota_i1[:, :])
                nc.vector.tensor_scalar(fr[:, :], fr[:, :], 1.0 / NB, None,
                                        op0=Op.mult)  # i/NB exact
                fr2 = pool.tile([NODES, 1], f32, tag="fr2")
                nc.vector.tensor_scalar(fr2[:, :], fr[:, :], MAGIC, -MAGIC,
                                        op0=Op.add, op1=Op.add)
                fg = pool.tile([NODES, 1], f32, tag="fg")
                nc.vector.tensor_tensor(fg[:, :], fr2[:, :], fr[:, :],
                                        Op.is_gt)
                nc.vector.tensor_sub(fr2[:, :], fr2[:, :], fg[:, :])  # floor
                nc.vector.tensor_scalar(fr2[:, :], fr2[:, :],
                                        -float(NB * S), None, op0=Op.mult)
                nc.vector.tensor_add(iota_isf[:, :], iota_isf[:, :],
                                     fr2[:, :])
            zsel = pool.tile([NODES, 1], f32, tag="zsel")
            nc.vector.tensor_scalar(zsel[:, :], n_keep, 0.0, None,
                                    op0=Op.is_equal)
            p3 = pool.tile([NODES, 8], f32, tag="p3")
            nc.vector.memset(p3[:, :], float(ZCOL))
            for qi, qv in enumerate(QUANTILES):
                zq = pool.tile([NODES, 1], f32, tag="zq")
                nc.vector.tensor_scalar(zq[:, :], n_keep, float(qv), None,
                                        op0=Op.mult)
                rq = pool.tile([NODES, 1], f32, tag="rq")
                nc.vector.tensor_scalar(rq[:, :], zq[:, :], MAGIC, -MAGIC,
                                        op0=Op.add, op1=Op.add)
                g1 = pool.tile([NODES, 1], f32, tag="g1")
                nc.vector.tensor_tensor(g1[:, :], rq[:, :], zq[:, :], Op.is_gt)
                nc.vector.tensor_sub(rq[:, :], rq[:, :], g1[:, :])  # floor
                nc.vector.tensor_tensor(g1[:, :], zq[:, :], rq[:, :], Op.is_gt)
                nc.vector.tensor_add(rq[:, :], rq[:, :], g1[:, :])  # ceil
                nc.vector.tensor_scalar(rq[:, :], rq[:, :], -1.0, 0.0,
                                        op0=Op.add, op1=Op.max)  # k_q
                # pos = i*S + k_q ; if n==0 -> ZCOL
                nc.vector.tensor_add(rq[:, :], rq[:, :], iota_isf[:, :])
                d1 = pool.tile([NODES, 1], f32, tag="d1")
                nc.vector.tensor_scalar(d1[:, :], rq[:, :], -1.0, float(ZCOL),
                                        op0=Op.mult, op1=Op.add)
                nc.vector.tensor_mul(d1[:, :], d1[:, :], zsel[:, :])
                nc.vector.tensor_add(p3[:, qi:qi + 1], rq[:, :], d1[:, :])

            # ---------------- J extraction: kept j indices per node
            iota_j = pool.tile([NODES, N], i32, tag="iota_j")
            nc.gpsimd.iota(iota_j[:, :], pattern=[[-1, N]], base=IOTA_BASE,
                           channel_multiplier=0)
            key = pool.tile([NODES, N], f32, tag="key")
            nc.vector.tensor_copy(key[:, :], iota_j[:, :])
            nc.vector.tensor_mul(key[:, :], key[:, :], keep[:, :])
            jt = pool.tile([NODES, S], f32, tag="jt")
            for r in range(R):
                mx = roll.tile([NODES, 8], f32, tag="mx")
                nc.vector.max(out=mx[:, :], in_=key[:, :])
                nc.vector.match_replace(out=key[:, :], in_to_replace=mx[:, :],
                                        in_values=key[:, :], imm_value=0.0)
                # key value encodes j: j = IOTA_BASE - key; exhausted rows
                # give key 0 -> IOTA_BASE, clamped to the SENT_COL sentinel.
                idxf = roll.tile([NODES, 8], f32, tag="idxf")
                nc.vector.tensor_scalar(idxf[:, :], mx[:, :], -1.0,
                                        float(IOTA_BASE), op0=Op.mult,
                                        op1=Op.add)
                nc.vector.tensor_scalar(jt[:, r * 8:(r + 1) * 8], idxf[:, :],
                                        float(SENT_COL), None, op0=Op.min)

            # J + P3 -> int16 -> DRAM -> wrapped idx layout (16-partition wrap,
            # replicated to the 8 gpsimd core groups)
            j16 = pool.tile([NODES, S], i16, tag="j16")
            nc.vector.tensor_copy(j16[:, :], jt[:, :])
            p16 = pool.tile([NODES, 8], i16, tag="p16")
            nc.vector.tensor_copy(p16[:, :], p3[:, :])
            jdram = dpool.tile([NODES, S], i16, tag="jdram")
            pdram = dpool.tile([NODES, 8], i16, tag="pdram")
            nc.sync.dma_start(jdram[:, :], j16[:, :])
            nc.sync.dma_start(pdram[:, :], p16[:, :])

            # wrapped idx list for the pick gather: unwrapped idx t = q*NB + i
            # lives at pdram addr i*8 + q; partition = t%16 = i%16.
            wrap_pick = pool.tile([128, (NODES * 8) // 16], i16, tag="wrap_pick")
            wp_src = bass.AP(pdram[:, :].tensor, 0,
                             [[8, 16], [1, 8], [8 * 16, NODES // 16]])
            for g in range(8):
                nc.sync.dma_start(wrap_pick[g * 16:(g + 1) * 16, :], wp_src)

            out_sb = pool.tile([NODES, D], f32, tag="out_sb")

            # xt_own / W / b loads
            xo = pool.tile([D, NODES], f32, tag="xo")
            nc.sync.dma_start(xo[:, :], xt_own[:, :])
            w_sb = pool.tile([D, D], f32, tag="w_sb")
            nc.sync.dma_start(w_sb[:, :], w_in[:, :])
            b_sb = pool.tile([D, 1], f32, tag="b_sb")
            nc.sync.dma_start(b_sb[:, :], b_in[:, :])
            wh = pool.tile([D, D], f32, tag="wh")
            scale_w = (1.0 - SELF_RATIO) if USE_SELF else 1.0
            nc.vector.tensor_scalar(wh[:, :], w_sb[:, :], float(scale_w), None,
                                    op0=Op.mult)
            pwt = psum.tile([D, D], f32, tag="ps")
            nc.tensor.transpose(pwt[:, :], wh[:, :], ident[:, :])
            whT = pool.tile([D, D], f32, tag="whT")  # [d, d'] = scale*W^T
            nc.scalar.copy(whT[:, :], pwt[:, :])

            s_mix = pool.tile([D, NODES], f32, tag="s_mix")

            for ib in range(NBATCH):
                nrows = slice(ib * NB, (ib + 1) * NB)
                # unwrapped idx t = i_local*S + s at jdram addr
                # (ib*NB + i_local)*S + s; partition = t%16 = s%16 (S%16==0).
                wrap_j = batchp.tile([128, (NB * S) // 16], i16, tag="wrap_j")
                wj_src = bass.AP(jdram[:, :].tensor, ib * NB * S,
                                 [[1, 16], [S, NB], [16, S // 16]])
                for g in range(8):
                    nc.sync.dma_start(wrap_j[g * 16:(g + 1) * 16, :], wj_src)

                slab = batchp.tile([D, SLABW + 8], f32, tag="slab")
                nc.vector.memset(slab[:, SLABW:], 0.0)
                nc.gpsimd.ap_gather(
                    out_ap=slab[:, :SLABW], in_ap=xt[:, :N + 1],
                    idxs_ap=wrap_j[:, :], channels=128, num_elems=N + 1, d=1,
                    num_idxs=NB * S)

                # ---- Batcher odd-even merge sort of each S-segment
                import bass_rust
                APc = bass.AP
                sl_t = slab[:, :].tensor
                part = list(slab[:, :].ap[0])
                scr = pool.tile([D, max_fam_pairs * NB], f32, tag="scr")
                scr_t = scr[:, :].tensor
                scr_part = list(scr[:, :].ap[0])
                for dist, fams in net:
                    for (s0, step, oc, rl) in fams:
                        dims = [part, [S, NB], [step, oc], [1, rl]]
                        lo = APc(sl_t, s0, [list(d) for d in dims])
                        hi = APc(sl_t, s0 + dist, [list(d) for d in dims])
                        tv = APc(scr_t, 0,
                                 [list(scr_part), [oc * rl, NB], [rl, oc],
                                  [1, rl]])
                        nc.scalar.copy(tv, lo)
                        nc.vector.tensor_tensor(lo, lo, hi, Op.min)
                        nc.vector.tensor_tensor(hi, tv, hi, Op.max)

                # ---- pick quantile slots
                picked = batchp.tile([D, NB * 8], f32, tag="picked")
                if NBATCH == 1:
                    wp = wrap_pick[:, :]
                else:
                    # unwrapped idx t = q*NB + i_local at pdram addr
                    # (ib*NB + i_local)*8 + q; partition = i_local%16.
                    wp_b = bass.AP(pdram[:, :].tensor, ib * NB * 8,
                                   [[8, 16], [1, 8], [8 * 16, NB // 16]])
                    wpt = batchp.tile([128, (NB * 8) // 16], i16, tag="wrap_pb")
                    for g in range(8):
                        nc.sync.dma_start(wpt[g * 16:(g + 1) * 16, :], wp_b)
                    wp = wpt[:, :]
                nc.gpsimd.ap_gather(
                    out_ap=picked[:, :], in_ap=slab[:, :SLABW + 8],
                    idxs_ap=wp, channels=128, num_elems=SLABW + 8, d=1,
                    num_idxs=NB * 8)

                agg = batchp.tile([D, NB], f32, tag="agg")
                nc.vector.tensor_add(agg[:, :], picked[:, 0:NB],
                                     picked[:, NB:2 * NB])
                nc.vector.tensor_add(agg[:, :], agg[:, :],
                                     picked[:, 2 * NB:3 * NB])
                wq = 1.0 / len(QUANTILES)
                if USE_SELF:
                    # s_mix = wq*agg + x ; out = (1-r)*s_mix @ W^T (+b +x)
                    nc.vector.scalar_tensor_tensor(
                        s_mix[:, nrows], agg[:, :], float(wq), xo[:, nrows],
                        op0=Op.mult, op1=Op.add)
                else:
                    nc.vector.tensor_scalar(s_mix[:, nrows], agg[:, :],
                                            float(wq), None, op0=Op.mult)

            # ---------------- output matmul: psum[d',i] = (s*W)[d',d]@s_mix
            pmm = psum.tile([D, NODES], f32, tag="ps")
            nc.tensor.matmul(pmm[:, :], whT[:, :], s_mix[:, :],
                             start=True, stop=True)
            out_di = pool.tile([D, NODES], f32, tag="out_di")
            if RESIDUAL:
                nc.vector.scalar_tensor_tensor(out_di[:, :], pmm[:, :],
                                               b_sb[:, :], xo[:, :],
                                               op0=Op.add, op1=Op.add)
            else:
                nc.vector.tensor_scalar(out_di[:, :], pmm[:, :], b_sb[:, :],
                                        None, op0=Op.add)
            pot = psum.tile([NODES, D], f32, tag="ps")
            nc.tensor.transpose(pot[:, :], out_di[:, :], ident[:, :])
            nc.scalar.copy(out_sb[:, :], pot[:, :])
            nc.sync.dma_start(out_t[:, :], out_sb[:, :])

    nc.finalize()
    return nc


def _get_program(N, D, S):
    key = (N, D, S)
    if key not in _PROGRAMS:
        _PROGRAMS[key] = _build_program(N, D, S)
    return _PROGRAMS[key]


# ---------------------------------------------------------------- host API
def kernel(x, A, W, b):
    try:
        import jax
        if jax.default_backend() != "axon" and any(
            p == "axon" for p in jax._src.xla_bridge.backends()
        ):
            jax.config.update("jax_platforms", "axon")
    except Exception:
        pass
    from concourse.bass_utils import run_bass_kernel_spmd

    x = np.ascontiguousarray(np.asarray(x, dtype=np.float32))
    A = np.ascontiguousarray(np.asarray(A, dtype=np.int32))
    W = np.ascontiguousarray(np.asarray(W, dtype=np.float32))
    b = np.ascontiguousarray(np.asarray(b, dtype=np.float32))
    N, D = x.shape
    NODES = N // N_CORES

    # S (slots per node) is a compile-time shape knob picked from the max
    # degree; used only for dispatch, never folded into the result.
    maxdeg = int((A != 0).sum(axis=1).max())
    S = max(16, int(np.ceil((maxdeg + 1) / 8.0)) * 8)
    S = min(S, N)
    nc = _get_program(N, D, S)

    in_maps = []
    for c in range(N_CORES):
        rows = slice(c * NODES, (c + 1) * NODES)
        in_maps.append({
            "a_slab": A[rows, :],
            "x_full": x,
            "xt_own": np.ascontiguousarray(x[rows, :].T),
            "w_in": W,
            "b_in": b.reshape(D, 1),
        })
    res = run_bass_kernel_spmd(nc, in_maps, core_ids=list(range(N_CORES)))
    out = np.concatenate([res.results[c]["out"] for c in range(N_CORES)],
                         axis=0)
    return out


# revision 13
# speedup vs baseline: 1.5687x; 1.5687x over previous
"""Trainium2 Bass kernel for nn_DilatedGCNLayer (masked per-node quantile
aggregation GCN layer).

Contract: kernel(**inputs) takes FULL inputs (x [1024,128] f32, A [1024,1024]
int32, W [128,128] f32, b [128] f32) and returns the FULL [1024,128] f32
output. Internally shards nodes across 8 NeuronCores (128 rows of x/A per
core), replicates x/W/b, runs one SPMD Bass program, and concatenates the
per-core output slabs.

Per-core algorithm (exact, no approximation):
  1. keep/rank phase on [nodes, N]: mask=A!=0, neighbor ranks via prefix
     scan, dilated removal of every skip-th neighbor, keep matrix, n_keep,
     quantile slot indices k_q = max(ceil(q*n_keep)-1, 0).
  2. compact neighbor lists: key = keep*(2048-j); repeated vector.max (8
     maxima/round) + max_index + match_replace extracts each node's kept
     j-indices in ascending-j order; pad slots point at a +FLT_MAX sentinel
     column of xT.
  3. gpsimd.ap_gather pulls xT[:, J] into a slab [128 d, nodes*S].
  4. per-node-segment sort along the free axis with a Batcher odd-even
     merge network (ascending comparators only, so the +FLT_MAX pad tail is
     stable); min/max on VectorE, left-copy on ScalarE, in place.
  5. second ap_gather picks the 3 quantile slots per node (zero sentinel
     for isolated nodes), average, mix with x, single PE matmul with
     0.5*W^T, add bias + residual, transpose back, store.
"""

import numpy as np

# module hyperparameters of the reference nn.Module
LAYER_IDX = 2
T_THRESH = 5
K_DIV = 2
EXP_DILATION = False
QUANTILES = (0.25, 0.5, 0.75)
SELF_RATIO = 0.5
USE_SELF = True
RESIDUAL = True

N_CORES = 8
BIG = float(np.finfo(np.float32).max)
MAGIC = 12582912.0  # 1.5 * 2**23: (v + MAGIC) - MAGIC == round-to-nearest-int


# ---------------------------------------------------------------- sort net
def _batcher_stages(n):
    stages = []
    p = 1
    while p < n:
        k = p
        while k >= 1:
            pairs = []
            j = k % p
            while j + k < n:
                for i in range(0, min(k, n - j - k)):
                    if (i + j) // (p * 2) == (i + j + k) // (p * 2):
                        pairs.append((i + j, i + j + k))
                j += 2 * k
            if pairs:
                stages.append(pairs)
            k //= 2
        p *= 2
    return stages


def _prune_stages(stages, n, keep_out):
    live = np.zeros(n, dtype=bool)
    live[:keep_out] = True
    out = []
    for pairs in reversed(stages):
        kept = []
        for lo, hi in pairs:
            if live[lo] or live[hi]:
                kept.append((lo, hi))
                live[lo] = True
                live[hi] = True
        if kept:
            out.append(kept)
    out.reverse()
    return out


def _pack_families(lefts):
    lefts = sorted(lefts)
    runs = []
    s = prev = lefts[0]
    for v in lefts[1:]:
        if v == prev + 1:
            prev = v
            continue
        runs.append((s, prev - s + 1))
        s = prev = v
    runs.append((s, prev - s + 1))
    fams = []
    i = 0
    while i < len(runs):
        s0, l0 = runs[i]
        j = i + 1
        if j < len(runs) and runs[j][1] == l0:
            step = runs[j][0] - s0
            while (j < len(runs) and runs[j][1] == l0
                   and runs[j][0] == s0 + (j - i) * step):
                j += 1
            fams.append((s0, step, j - i, l0))
        else:
            fams.append((s0, 1, 1, l0))
        i = j
    return fams


def _net_for(n, keep_out):
    stages = _batcher_stages(n)
    if keep_out < n:
        stages = _prune_stages(stages, n, keep_out)
    packed = []
    for pairs in stages:
        dist = pairs[0][1] - pairs[0][0]
        assert all(hi - lo == dist for lo, hi in pairs)
        packed.append((dist, _pack_families([lo for lo, _ in pairs])))
    return packed


# ---------------------------------------------------------------- program
_PROGRAMS = {}


def _build_program(N, D, S):
    import concourse.bass as bass
    import concourse.bacc as bacc
    import concourse.mybir as mybir
    from concourse.tile import TileContext
    from concourse.masks import make_identity

    f32 = mybir.dt.float32
    i32 = mybir.dt.int32
    i16 = mybir.dt.int16
    u32 = mybir.dt.uint32
    Op = mybir.AluOpType

    NODES = N // N_CORES
    NB = NODES
    while NB * S > 16384:
        NB //= 2
    import os
    # split into batches so the gpsimd gather of batch i+1 overlaps the
    # DVE sort of batch i (head-to-head measured ~225us faster than a
    # single batch)
    split = int(os.environ.get("DGCN_SPLIT", "4"))
    while split > 1 and (NODES % split or (NODES // split) % 16
                         or (NODES // split) < 16):
        split -= 1
    if NB == NODES:
        NB = NODES // split
    NBATCH = NODES // NB
    R = S // 8                      # max8 extraction rounds
    SENT_COL = N                    # xT column holding +BIG
    SLABW = NB * S                  # slab real width
    ZCOL = SLABW                    # slab column holding 0.0 (pick sentinel)
    IOTA_BASE = 2 * N               # key = keep * (IOTA_BASE - j) > 0

    nc = bacc.Bacc(None, target_bir_lowering=False)

    a_slab = nc.dram_tensor("a_slab", [NODES, N], i32, kind="ExternalInput")
    x_full = nc.dram_tensor("x_full", [N, D], f32, kind="ExternalInput")
    xt_own = nc.dram_tensor("xt_own", [D, NODES], f32, kind="ExternalInput")
    w_in = nc.dram_tensor("w_in", [D, D], f32, kind="ExternalInput")
    b_in = nc.dram_tensor("b_in", [D, 1], f32, kind="ExternalInput")
    out_t = nc.dram_tensor("out", [NODES, D], f32, kind="ExternalOutput")

    net = _net_for(S, int(np.ceil(0.75 * S)))
    max_fam_pairs = max(oc * rl for _, fams in net for (_, _, oc, rl) in fams)

    with TileContext(nc) as tc:
        with (
            tc.tile_pool(name="main", bufs=1) as pool,
            tc.tile_pool(name="roll", bufs=3) as roll,
            tc.tile_pool(name="batch", bufs=2) as batchp,
            tc.tile_pool(name="psum", bufs=2, space="PSUM") as psum,
            tc.tile_pool(name="dram", bufs=1, space="DRAM") as dpool,
        ):
            # ---------------- identity for PE transposes
            ident = pool.tile([128, 128], f32, tag="ident")
            make_identity(nc, ident[:, :])

            # ---------------- xT: [D, N+2]; col N..N+2 = +BIG sentinel
            xt = pool.tile([D, N + 2], f32, tag="xt")
            nc.vector.memset(xt[:, N:], BIG)
            for c in range(N // 128):
                xc = roll.tile([128, D], f32, tag="xc")
                nc.sync.dma_start(xc[:, :], x_full[c * 128:(c + 1) * 128, :])
                pt = psum.tile([D, 128], f32, tag="ps")
                nc.tensor.transpose(pt[:, :], xc[:, :], ident[:, :])
                nc.scalar.copy(xt[:, c * 128:(c + 1) * 128], pt[:, :])

            # ---------------- keep / rank phase  [NODES, N]
            a_sb = pool.tile([NODES, N], i32, tag="a_sb")
            nc.sync.dma_start(a_sb[:, :], a_slab[:, :])
            maskf = pool.tile([NODES, N], f32, tag="maskf")
            nc.vector.tensor_scalar(maskf[:, :], a_sb[:, :], 0, None,
                                    op0=Op.not_equal)
            zeros_row = pool.tile([NODES, N], f32, tag="zeros_row")
            nc.vector.memset(zeros_row[:, :], 0.0)
            # rank1[i,j] = inclusive prefix sum of mask = (# nbrs <= j)
            rank1 = pool.tile([NODES, N], f32, tag="rank1")
            nc.vector.tensor_tensor_scan(rank1[:, :], maskf[:, :],
                                         zeros_row[:, :], 0.0,
                                         op0=Op.add, op1=Op.max)
            num_nbr = rank1[:, N - 1:N]

            # skip = where(n > T, ceil(n/K), 1)   (K=2)  [NODES,1]
            half = pool.tile([NODES, 1], f32, tag="half")
            nc.vector.tensor_scalar(half[:, :], num_nbr, 1.0, 0.5,
                                    op0=Op.add, op1=Op.mult)  # (n+1)/2
            rnd = pool.tile([NODES, 1], f32, tag="rnd")
            nc.vector.tensor_scalar(rnd[:, :], half[:, :], MAGIC, -MAGIC,
                                    op0=Op.add, op1=Op.add)
            gt = pool.tile([NODES, 1], f32, tag="gt")
            nc.vector.tensor_tensor(gt[:, :], rnd[:, :], half[:, :], Op.is_gt)
            m_t = pool.tile([NODES, 1], f32, tag="m_t")
            nc.vector.tensor_sub(m_t[:, :], rnd[:, :], gt[:, :])  # floor=ceil(n/2)
            sel = pool.tile([NODES, 1], f32, tag="sel")
            nc.vector.tensor_scalar(sel[:, :], num_nbr, float(T_THRESH), None,
                                    op0=Op.is_gt)
            mm1 = pool.tile([NODES, 1], f32, tag="mm1")
            nc.vector.tensor_scalar(mm1[:, :], m_t[:, :], -1.0, None, op0=Op.add)
            skipv = pool.tile([NODES, 1], f32, tag="skipv")
            nc.vector.scalar_tensor_tensor(skipv[:, :], mm1[:, :], 1.0,
                                           sel[:, :], op0=Op.mult, op1=Op.mult)
            nc.vector.tensor_scalar(skipv[:, :], skipv[:, :], 1.0, None,
                                    op0=Op.add)  # 1 + sel*(m-1)
            recip = pool.tile([NODES, 1], f32, tag="recip")
            nc.vector.reciprocal(recip[:, :], skipv[:, :])

            # keep = mask & ((rank1 % skip) != 0)
            qq = pool.tile([NODES, N], f32, tag="qq")
            nc.vector.tensor_scalar(qq[:, :], rank1[:, :], recip[:, :], None,
                                    op0=Op.mult)
            nc.vector.tensor_scalar(qq[:, :], qq[:, :], MAGIC, -MAGIC,
                                    op0=Op.add, op1=Op.add)  # round(rank/skip)
            prod = pool.tile([NODES, N], f32, tag="prod")
            nc.vector.scalar_tensor_tensor(prod[:, :], qq[:, :], skipv[:, :],
                                           rank1[:, :], op0=Op.mult,
                                           op1=Op.subtract)
            is0 = pool.tile([NODES, N], f32, tag="is0")
            nc.vector.tensor_scalar(is0[:, :], prod[:, :], 0.0, None,
                                    op0=Op.is_equal)
            keep = pool.tile([NODES, N], f32, tag="keep")
            nc.vector.tensor_scalar(is0[:, :], is0[:, :], -1.0, 1.0,
                                    op0=Op.mult, op1=Op.add)  # 1-is0
            nc.vector.tensor_mul(keep[:, :], maskf[:, :], is0[:, :])

            csk = pool.tile([NODES, N], f32, tag="csk")
            nc.vector.tensor_tensor_scan(csk[:, :], keep[:, :], zeros_row[:, :],
                                         0.0, op0=Op.add, op1=Op.max)
            n_keep = csk[:, N - 1:N]

            # quantile slot positions P3[:, q] = i*S + max(ceil(q*n)-1, 0)
            # (q-padded to 8, sentinel ZCOL for pads and isolated nodes)
            # batch-local slab base (i % NB) * S, computed exactly:
            # i/NB is dyadic (NB power of two), floor via the magic-round
            # trick, then base = i*S - floor(i/NB)*NB*S.
            iota_is = pool.tile([NODES, 1], i32, tag="iota_is")
            nc.gpsimd.iota(iota_is[:, :], pattern=[[0, 1]], base=0,
                           channel_multiplier=S)
            iota_isf = pool.tile([NODES, 1], f32, tag="iota_isf")
            nc.vector.tensor_copy(iota_isf[:, :], iota_is[:, :])
            if NBATCH > 1:
                iota_i1 = pool.tile([NODES, 1], i32, tag="iota_i1")
                nc.gpsimd.iota(iota_i1[:, :], pattern=[[0, 1]], base=0,
                               channel_multiplier=1)
                fr = pool.tile([NODES, 1], f32, tag="fr")
                nc.vector.tensor_copy(fr[:, :], iota_i1[:, :])
                nc.vector.tensor_scalar(fr[:, :], fr[:, :], 1.0 / NB, None,
                                        op0=Op.mult)  # i/NB exact
                fr2 = pool.tile([NODES, 1], f32, tag="fr2")
                nc.vector.tensor_scalar(fr2[:, :], fr[:, :], MAGIC, -MAGIC,
                                        op0=Op.add, op1=Op.add)
                fg = pool.tile([NODES, 1], f32, tag="fg")
                nc.vector.tensor_tensor(fg[:, :], fr2[:, :], fr[:, :],
                                        Op.is_gt)
                nc.vector.tensor_sub(fr2[:, :], fr2[:, :], fg[:, :])  # floor
                nc.vector.tensor_scalar(fr2[:, :], fr2[:, :],
                                        -float(NB * S), None, op0=Op.mult)
                nc.vector.tensor_add(iota_isf[:, :], iota_isf[:, :],
                                     fr2[:, :])
            zsel = pool.tile([NODES, 1], f32, tag="zsel")
            nc.vector.tensor_scalar(zsel[:, :], n_keep, 0.0, None,
                                    op0=Op.is_equal)
            p3 = pool.tile([NODES, 8], f32, tag="p3")
            nc.vector.memset(p3[:, :], float(ZCOL))
            for qi, qv in enumerate(QUANTILES):
                zq = pool.tile([NODES, 1], f32, tag="zq")
                nc.vector.tensor_scalar(zq[:, :], n_keep, float(qv), None,
                                        op0=Op.mult)
                rq = pool.tile([NODES, 1], f32, tag="rq")
                nc.vector.tensor_scalar(rq[:, :], zq[:, :], MAGIC, -MAGIC,
                                        op0=Op.add, op1=Op.add)
                g1 = pool.tile([NODES, 1], f32, tag="g1")
                nc.vector.tensor_tensor(g1[:, :], rq[:, :], zq[:, :], Op.is_gt)
                nc.vector.tensor_sub(rq[:, :], rq[:, :], g1[:, :])  # floor
                nc.vector.tensor_tensor(g1[:, :], zq[:, :], rq[:, :], Op.is_gt)
                nc.vector.tensor_add(rq[:, :], rq[:, :], g1[:, :])  # ceil
                nc.vector.tensor_scalar(rq[:, :], rq[:, :], -1.0, 0.0,
                                        op0=Op.add, op1=Op.max)  # k_q
                # pos = i*S + k_q ; if n==0 -> ZCOL
                nc.vector.tensor_add(rq[:, :], rq[:, :], iota_isf[:, :])
                d1 = pool.tile([NODES, 1], f32, tag="d1")
                nc.vector.tensor_scalar(d1[:, :], rq[:, :], -1.0, float(ZCOL),
                                        op0=Op.mult, op1=Op.add)
                nc.vector.tensor_mul(d1[:, :], d1[:, :], zsel[:, :])
                nc.vector.tensor_add(p3[:, qi:qi + 1], rq[:, :], d1[:, :])

            # ---------------- J extraction: kept j indices per node
            iota_j = pool.tile([NODES, N], i32, tag="iota_j")
            nc.gpsimd.iota(iota_j[:, :], pattern=[[-1, N]], base=IOTA_BASE,
                           channel_multiplier=0)
            key = pool.tile([NODES, N], f32, tag="key")
            nc.vector.tensor_copy(key[:, :], iota_j[:, :])
            nc.vector.tensor_mul(key[:, :], key[:, :], keep[:, :])
            jt = pool.tile([NODES, S], f32, tag="jt")
            for r in range(R):
                mx = roll.tile([NODES, 8], f32, tag="mx")
                nc.vector.max(out=mx[:, :], in_=key[:, :])
                nc.vector.match_replace(out=key[:, :], in_to_replace=mx[:, :],
                                        in_values=key[:, :], imm_value=0.0)
                # key value encodes j: j = IOTA_BASE - key; exhausted rows
                # give key 0 -> IOTA_BASE, clamped to the SENT_COL sentinel.
                idxf = roll.tile([NODES, 8], f32, tag="idxf")
                nc.vector.tensor_scalar(idxf[:, :], mx[:, :], -1.0,
                                        float(IOTA_BASE), op0=Op.mult,
                                        op1=Op.add)
                nc.vector.tensor_scalar(jt[:, r * 8:(r + 1) * 8], idxf[:, :],
                                        float(SENT_COL), None, op0=Op.min)

            # J + P3 -> int16 -> DRAM -> wrapped idx layout (16-partition wrap,
            # replicated to the 8 gpsimd core groups)
            j16 = pool.tile([NODES, S], i16, tag="j16")
            nc.vector.tensor_copy(j16[:, :], jt[:, :])
            p16 = pool.tile([NODES, 8], i16, tag="p16")
            nc.vector.tensor_copy(p16[:, :], p3[:, :])
            jdram = dpool.tile([NODES, S], i16, tag="jdram")
            pdram = dpool.tile([NODES, 8], i16, tag="pdram")
            nc.sync.dma_start(jdram[:, :], j16[:, :])
            nc.sync.dma_start(pdram[:, :], p16[:, :])

            # wrapped idx list for the pick gather: unwrapped idx t = q*NB + i
            # lives at pdram addr i*8 + q; partition = t%16 = i%16.
            wrap_pick = pool.tile([128, (NODES * 8) // 16], i16, tag="wrap_pick")
            wp_src = bass.AP(pdram[:, :].tensor, 0,
                             [[8, 16], [1, 8], [8 * 16, NODES // 16]])
            for g in range(8):
                nc.sync.dma_start(wrap_pick[g * 16:(g + 1) * 16, :], wp_src)

            out_sb = pool.tile([NODES, D], f32, tag="out_sb")

            # xt_own / W / b loads
            xo = pool.tile([D, NODES], f32, tag="xo")
            nc.sync.dma_start(xo[:, :], xt_own[:, :])
            w_sb = pool.tile([D, D], f32, tag="w_sb")
            nc.sync.dma_start(w_sb[:, :], w_in[:, :])
            b_sb = pool.tile([D, 1], f32, tag="b_sb")
            nc.sync.dma_start(b_sb[:, :], b_in[:, :])
            wh = pool.tile([D, D], f32, tag="wh")
            scale_w = (1.0 - SELF_RATIO) if USE_SELF else 1.0
            nc.vector.tensor_scalar(wh[:, :], w_sb[:, :], float(scale_w), None,
                                    op0=Op.mult)
            pwt = psum.tile([D, D], f32, tag="ps")
            nc.tensor.transpose(pwt[:, :], wh[:, :], ident[:, :])
            whT = pool.tile([D, D], f32, tag="whT")  # [d, d'] = scale*W^T
            nc.scalar.copy(whT[:, :], pwt[:, :])

            s_mix = pool.tile([D, NODES], f32, tag="s_mix")

            for ib in range(NBATCH):
                nrows = slice(ib * NB, (ib + 1) * NB)
                # unwrapped idx t = i_local*S + s at jdram addr
                # (ib*NB + i_local)*S + s; partition = t%16 = s%16 (S%16==0).
                wrap_j = batchp.tile([128, (NB * S) // 16], i16, tag="wrap_j")
                wj_src = bass.AP(jdram[:, :].tensor, ib * NB * S,
                                 [[1, 16], [S, NB], [16, S // 16]])
                for g in range(8):
                    nc.sync.dma_start(wrap_j[g * 16:(g + 1) * 16, :], wj_src)

                slab = batchp.tile([D, SLABW + 8], f32, tag="slab")
                nc.vector.memset(slab[:, SLABW:], 0.0)
                nc.gpsimd.ap_gather(
                    out_ap=slab[:, :SLABW], in_ap=xt[:, :N + 1],
                    idxs_ap=wrap_j[:, :], channels=128, num_elems=N + 1, d=1,
                    num_idxs=NB * S)

                # ---- Batcher odd-even merge sort of each S-segment
                import bass_rust
                APc = bass.AP
                sl_t = slab[:, :].tensor
                part = list(slab[:, :].ap[0])
                scr = pool.tile([D, max_fam_pairs * NB], f32, tag="scr")
                scr_t = scr[:, :].tensor
                scr_part = list(scr[:, :].ap[0])
                for dist, fams in net:
                    for (s0, step, oc, rl) in fams:
                        dims = [part, [S, NB], [step, oc], [1, rl]]
                        lo = APc(sl_t, s0, [list(d) for d in dims])
                        hi = APc(sl_t, s0 + dist, [list(d) for d in dims])
                        tv = APc(scr_t, 0,
                                 [list(scr_part), [oc * rl, NB], [rl, oc],
                                  [1, rl]])
                        nc.scalar.copy(tv, lo)
                        nc.vector.tensor_tensor(lo, lo, hi, Op.min)
                        nc.vector.tensor_tensor(hi, tv, hi, Op.max)

                # ---- pick quantile slots
                picked = batchp.tile([D, NB * 8], f32, tag="picked")
                if NBATCH == 1:
                    wp = wrap_pick[:, :]
                else:
                    # unwrapped idx t = q*NB + i_local at pdram addr
                    # (ib*NB + i_local)*8 + q; partition = i_local%16.
                    wp_b = bass.AP(pdram[:, :].tensor, ib * NB * 8,
                                   [[8, 16], [1, 8], [8 * 16, NB // 16]])
                    wpt = batchp.tile([128, (NB * 8) // 16], i16, tag="wrap_pb")
                    for g in range(8):
                        nc.sync.dma_start(wpt[g * 16:(g + 1) * 16, :], wp_b)
                    wp = wpt[:, :]
                nc.gpsimd.ap_gather(
                    out_ap=picked[:, :], in_ap=slab[:, :SLABW + 8],
                    idxs_ap=wp, channels=128, num_elems=SLABW + 8, d=1,
                    num_idxs=NB * 8)

                agg = batchp.tile([D, NB], f32, tag="agg")
                nc.vector.tensor_add(agg[:, :], picked[:, 0:NB],
                                     picked[:, NB:2 * NB])
                nc.vector.tensor_add(agg[:, :], agg[:, :],
                                     picked[:, 2 * NB:3 * NB])
                wq = 1.0 / len(QUANTILES)
                if USE_SELF:
                    # s_mix = wq*agg + x ; out = (1-r)*s_mix @ W^T (+b +x)
                    nc.vector.scalar_tensor_tensor(
                        s_mix[:, nrows], agg[:, :], float(wq), xo[:, nrows],
                        op0=Op.mult, op1=Op.add)
                else:
                    nc.vector.tensor_scalar(s_mix[:, nrows], agg[:, :],
                                            float(wq), None, op0=Op.mult)

            # ---------------- output matmul: psum[d',i] = (s*W)[d',d]@s_mix
            pmm = psum.tile([D, NODES], f32, tag="ps")
            nc.tensor.matmul(pmm[:, :], whT[:, :], s_mix[:, :],
                             start=True, stop=True)
            out_di = pool.tile([D, NODES], f32, tag="out_di")
            if RESIDUAL:
                nc.vector.scalar_tensor_tensor(out_di[:, :], pmm[:, :],
                                               b_sb[:, :], xo[:, :],
                                               op0=Op.add, op1=Op.add)
            else:
                nc.vector.tensor_scalar(out_di[:, :], pmm[:, :], b_sb[:, :],
                                        None, op0=Op.add)
            pot = psum.tile([NODES, D], f32, tag="ps")
            nc.tensor.transpose(pot[:, :], out_di[:, :], ident[:, :])
            nc.scalar.copy(out_sb[:, :], pot[:, :])
            nc.sync.dma_start(out_t[:, :], out_sb[:, :])

    nc.finalize()
    return nc


def _get_program(N, D, S):
    key = (N, D, S)
    if key not in _PROGRAMS:
        _PROGRAMS[key] = _build_program(N, D, S)
    return _PROGRAMS[key]


# ---------------------------------------------------------------- host API
def kernel(x, A, W, b):
    try:
        import jax
        if jax.default_backend() != "axon" and any(
            p == "axon" for p in jax._src.xla_bridge.backends()
        ):
            jax.config.update("jax_platforms", "axon")
    except Exception:
        pass
    from concourse.bass_utils import run_bass_kernel_spmd

    x = np.ascontiguousarray(np.asarray(x, dtype=np.float32))
    A = np.ascontiguousarray(np.asarray(A, dtype=np.int32))
    W = np.ascontiguousarray(np.asarray(W, dtype=np.float32))
    b = np.ascontiguousarray(np.asarray(b, dtype=np.float32))
    N, D = x.shape
    NODES = N // N_CORES

    # S (slots per node) is a compile-time shape knob picked from the max
    # degree; used only for dispatch, never folded into the result.
    maxdeg = int((A != 0).sum(axis=1).max())
    S = max(16, int(np.ceil((maxdeg + 1) / 8.0)) * 8)
    S = min(S, N)
    nc = _get_program(N, D, S)

    in_maps = []
    for c in range(N_CORES):
        rows = slice(c * NODES, (c + 1) * NODES)
        in_maps.append({
            "a_slab": A[rows, :],
            "x_full": x,
            "xt_own": np.ascontiguousarray(x[rows, :].T),
            "w_in": W,
            "b_in": b.reshape(D, 1),
        })
    res = run_bass_kernel_spmd(nc, in_maps, core_ids=list(range(N_CORES)))
    out = np.concatenate([res.results[c]["out"] for c in range(N_CORES)],
                         axis=0)
    return out


# revision 14
# speedup vs baseline: 1.6184x; 1.0317x over previous
"""Trainium2 Bass kernel for nn_DilatedGCNLayer (masked per-node quantile
aggregation GCN layer).

Contract: kernel(**inputs) takes FULL inputs (x [1024,128] f32, A [1024,1024]
int32, W [128,128] f32, b [128] f32) and returns the FULL [1024,128] f32
output. Internally shards nodes across 8 NeuronCores (128 rows of x/A per
core), replicates x/W/b, runs one SPMD Bass program, and concatenates the
per-core output slabs.

Per-core algorithm (exact, no approximation):
  1. keep/rank phase on [nodes, N]: mask=A!=0, neighbor ranks via prefix
     scan, dilated removal of every skip-th neighbor, keep matrix, n_keep,
     quantile slot indices k_q = max(ceil(q*n_keep)-1, 0).
  2. compact neighbor lists: key = keep*(2048-j); repeated vector.max (8
     maxima/round) + max_index + match_replace extracts each node's kept
     j-indices in ascending-j order; pad slots point at a +FLT_MAX sentinel
     column of xT.
  3. gpsimd.ap_gather pulls xT[:, J] into a slab [128 d, nodes*S].
  4. per-node-segment sort along the free axis with a Batcher odd-even
     merge network (ascending comparators only, so the +FLT_MAX pad tail is
     stable); min/max on VectorE, left-copy on ScalarE, in place.
  5. second ap_gather picks the 3 quantile slots per node (zero sentinel
     for isolated nodes), average, mix with x, single PE matmul with
     0.5*W^T, add bias + residual, transpose back, store.
"""

import numpy as np

# module hyperparameters of the reference nn.Module
LAYER_IDX = 2
T_THRESH = 5
K_DIV = 2
EXP_DILATION = False
QUANTILES = (0.25, 0.5, 0.75)
SELF_RATIO = 0.5
USE_SELF = True
RESIDUAL = True

N_CORES = 8
BIG = float(np.finfo(np.float32).max)
MAGIC = 12582912.0  # 1.5 * 2**23: (v + MAGIC) - MAGIC == round-to-nearest-int


# ---------------------------------------------------------------- sort net
def _batcher_stages(n):
    stages = []
    p = 1
    while p < n:
        k = p
        while k >= 1:
            pairs = []
            j = k % p
            while j + k < n:
                for i in range(0, min(k, n - j - k)):
                    if (i + j) // (p * 2) == (i + j + k) // (p * 2):
                        pairs.append((i + j, i + j + k))
                j += 2 * k
            if pairs:
                stages.append(pairs)
            k //= 2
        p *= 2
    return stages


def _prune_stages(stages, n, keep_out):
    live = np.zeros(n, dtype=bool)
    live[:keep_out] = True
    out = []
    for pairs in reversed(stages):
        kept = []
        for lo, hi in pairs:
            if live[lo] or live[hi]:
                kept.append((lo, hi))
                live[lo] = True
                live[hi] = True
        if kept:
            out.append(kept)
    out.reverse()
    return out


def _pack_families(lefts):
    lefts = sorted(lefts)
    runs = []
    s = prev = lefts[0]
    for v in lefts[1:]:
        if v == prev + 1:
            prev = v
            continue
        runs.append((s, prev - s + 1))
        s = prev = v
    runs.append((s, prev - s + 1))
    fams = []
    i = 0
    while i < len(runs):
        s0, l0 = runs[i]
        j = i + 1
        if j < len(runs) and runs[j][1] == l0:
            step = runs[j][0] - s0
            while (j < len(runs) and runs[j][1] == l0
                   and runs[j][0] == s0 + (j - i) * step):
                j += 1
            fams.append((s0, step, j - i, l0))
        else:
            fams.append((s0, 1, 1, l0))
        i = j
    return fams


def _net_for(n, keep_out):
    stages = _batcher_stages(n)
    if keep_out < n:
        stages = _prune_stages(stages, n, keep_out)
    packed = []
    for pairs in stages:
        dist = pairs[0][1] - pairs[0][0]
        assert all(hi - lo == dist for lo, hi in pairs)
        packed.append((dist, _pack_families([lo for lo, _ in pairs])))
    return packed


# ---------------------------------------------------------------- program
_PROGRAMS = {}


def _build_program(N, D, S):
    import concourse.bass as bass
    import concourse.bacc as bacc
    import concourse.mybir as mybir
    from concourse.tile import TileContext
    from concourse.masks import make_identity

    f32 = mybir.dt.float32
    i32 = mybir.dt.int32
    i16 = mybir.dt.int16
    u32 = mybir.dt.uint32
    Op = mybir.AluOpType

    NODES = N // N_CORES
    NB = NODES
    while NB * S > 16384:
        NB //= 2
    import os
    # split into batches so the gpsimd gather of batch i+1 overlaps the
    # DVE sort of batch i (head-to-head measured ~225us faster than a
    # single batch)
    split = int(os.environ.get("DGCN_SPLIT", "4"))
    while split > 1 and (NODES % split or (NODES // split) % 16
                         or (NODES // split) < 16):
        split -= 1
    if NB == NODES:
        NB = NODES // split
    NBATCH = NODES // NB
    R = S // 8                      # max8 extraction rounds
    SENT_COL = N                    # xT column holding +BIG
    SLABW = NB * S                  # slab real width
    ZCOL = SLABW                    # slab column holding 0.0 (pick sentinel)
    IOTA_BASE = 2 * N               # key = keep * (IOTA_BASE - j) > 0

    nc = bacc.Bacc(None, target_bir_lowering=False)

    a_slab = nc.dram_tensor("a_slab", [NODES, N], i32, kind="ExternalInput")
    x_full = nc.dram_tensor("x_full", [N, D], f32, kind="ExternalInput")
    xt_own = nc.dram_tensor("xt_own", [D, NODES], f32, kind="ExternalInput")
    w_in = nc.dram_tensor("w_in", [D, D], f32, kind="ExternalInput")
    b_in = nc.dram_tensor("b_in", [D, 1], f32, kind="ExternalInput")
    out_t = nc.dram_tensor("out", [NODES, D], f32, kind="ExternalOutput")

    net = _net_for(S, int(np.ceil(0.75 * S)))
    max_fam_pairs = max(oc * rl for _, fams in net for (_, _, oc, rl) in fams)

    with TileContext(nc) as tc:
        with (
            tc.tile_pool(name="main", bufs=1) as pool,
            tc.tile_pool(name="roll", bufs=3) as roll,
            tc.tile_pool(name="batch", bufs=2) as batchp,
            tc.tile_pool(name="psum", bufs=2, space="PSUM") as psum,
            tc.tile_pool(name="dram", bufs=1, space="DRAM") as dpool,
        ):
            # ---------------- identity for PE transposes
            ident = pool.tile([128, 128], f32, tag="ident")
            make_identity(nc, ident[:, :])

            # ---------------- xT: [D, N+2]; col N..N+2 = +BIG sentinel
            xt = pool.tile([D, N + 2], f32, tag="xt")
            nc.vector.memset(xt[:, N:], BIG)
            for c in range(N // 128):
                xc = roll.tile([128, D], f32, tag="xc")
                nc.sync.dma_start(xc[:, :], x_full[c * 128:(c + 1) * 128, :])
                pt = psum.tile([D, 128], f32, tag="ps")
                nc.tensor.transpose(pt[:, :], xc[:, :], ident[:, :])
                nc.scalar.copy(xt[:, c * 128:(c + 1) * 128], pt[:, :])

            # ---------------- keep / rank phase  [NODES, N]
            a_sb = pool.tile([NODES, N], i32, tag="a_sb")
            nc.sync.dma_start(a_sb[:, :], a_slab[:, :])
            maskf = pool.tile([NODES, N], f32, tag="maskf")
            nc.vector.tensor_scalar(maskf[:, :], a_sb[:, :], 0, None,
                                    op0=Op.not_equal)
            zeros_row = pool.tile([NODES, N], f32, tag="zeros_row")
            nc.vector.memset(zeros_row[:, :], 0.0)
            # rank1[i,j] = inclusive prefix sum of mask = (# nbrs <= j)
            rank1 = pool.tile([NODES, N], f32, tag="rank1")
            nc.vector.tensor_tensor_scan(rank1[:, :], maskf[:, :],
                                         zeros_row[:, :], 0.0,
                                         op0=Op.add, op1=Op.max)
            num_nbr = rank1[:, N - 1:N]

            # skip = where(n > T, ceil(n/K), 1)   (K=2)  [NODES,1]
            half = pool.tile([NODES, 1], f32, tag="half")
            nc.vector.tensor_scalar(half[:, :], num_nbr, 1.0, 0.5,
                                    op0=Op.add, op1=Op.mult)  # (n+1)/2
            rnd = pool.tile([NODES, 1], f32, tag="rnd")
            nc.vector.tensor_scalar(rnd[:, :], half[:, :], MAGIC, -MAGIC,
                                    op0=Op.add, op1=Op.add)
            gt = pool.tile([NODES, 1], f32, tag="gt")
            nc.vector.tensor_tensor(gt[:, :], rnd[:, :], half[:, :], Op.is_gt)
            m_t = pool.tile([NODES, 1], f32, tag="m_t")
            nc.vector.tensor_sub(m_t[:, :], rnd[:, :], gt[:, :])  # floor=ceil(n/2)
            sel = pool.tile([NODES, 1], f32, tag="sel")
            nc.vector.tensor_scalar(sel[:, :], num_nbr, float(T_THRESH), None,
                                    op0=Op.is_gt)
            mm1 = pool.tile([NODES, 1], f32, tag="mm1")
            nc.vector.tensor_scalar(mm1[:, :], m_t[:, :], -1.0, None, op0=Op.add)
            skipv = pool.tile([NODES, 1], f32, tag="skipv")
            nc.vector.scalar_tensor_tensor(skipv[:, :], mm1[:, :], 1.0,
                                           sel[:, :], op0=Op.mult, op1=Op.mult)
            nc.vector.tensor_scalar(skipv[:, :], skipv[:, :], 1.0, None,
                                    op0=Op.add)  # 1 + sel*(m-1)
            recip = pool.tile([NODES, 1], f32, tag="recip")
            nc.vector.reciprocal(recip[:, :], skipv[:, :])

            # keep = mask & ((rank1 % skip) != 0)
            qq = pool.tile([NODES, N], f32, tag="qq")
            nc.vector.tensor_scalar(qq[:, :], rank1[:, :], recip[:, :], None,
                                    op0=Op.mult)
            nc.vector.tensor_scalar(qq[:, :], qq[:, :], MAGIC, -MAGIC,
                                    op0=Op.add, op1=Op.add)  # round(rank/skip)
            prod = pool.tile([NODES, N], f32, tag="prod")
            nc.vector.scalar_tensor_tensor(prod[:, :], qq[:, :], skipv[:, :],
                                           rank1[:, :], op0=Op.mult,
                                           op1=Op.subtract)
            is0 = pool.tile([NODES, N], f32, tag="is0")
            nc.vector.tensor_scalar(is0[:, :], prod[:, :], 0.0, None,
                                    op0=Op.is_equal)
            keep = pool.tile([NODES, N], f32, tag="keep")
            nc.vector.tensor_scalar(is0[:, :], is0[:, :], -1.0, 1.0,
                                    op0=Op.mult, op1=Op.add)  # 1-is0
            nc.vector.tensor_mul(keep[:, :], maskf[:, :], is0[:, :])

            csk = pool.tile([NODES, N], f32, tag="csk")
            nc.vector.tensor_tensor_scan(csk[:, :], keep[:, :], zeros_row[:, :],
                                         0.0, op0=Op.add, op1=Op.max)
            n_keep = csk[:, N - 1:N]

            # quantile slot positions P3[:, q] = i*S + max(ceil(q*n)-1, 0)
            # (q-padded to 8, sentinel ZCOL for pads and isolated nodes)
            # batch-local slab base (i % NB) * S, computed exactly:
            # i/NB is dyadic (NB power of two), floor via the magic-round
            # trick, then base = i*S - floor(i/NB)*NB*S.
            iota_is = pool.tile([NODES, 1], i32, tag="iota_is")
            nc.gpsimd.iota(iota_is[:, :], pattern=[[0, 1]], base=0,
                           channel_multiplier=S)
            iota_isf = pool.tile([NODES, 1], f32, tag="iota_isf")
            nc.vector.tensor_copy(iota_isf[:, :], iota_is[:, :])
            if NBATCH > 1:
                iota_i1 = pool.tile([NODES, 1], i32, tag="iota_i1")
                nc.gpsimd.iota(iota_i1[:, :], pattern=[[0, 1]], base=0,
                               channel_multiplier=1)
                fr = pool.tile([NODES, 1], f32, tag="fr")
                nc.vector.tensor_copy(fr[:, :], iota_i1[:, :])
                nc.vector.tensor_scalar(fr[:, :], fr[:, :], 1.0 / NB, None,
                                        op0=Op.mult)  # i/NB exact
                fr2 = pool.tile([NODES, 1], f32, tag="fr2")
                nc.vector.tensor_scalar(fr2[:, :], fr[:, :], MAGIC, -MAGIC,
                                        op0=Op.add, op1=Op.add)
                fg = pool.tile([NODES, 1], f32, tag="fg")
                nc.vector.tensor_tensor(fg[:, :], fr2[:, :], fr[:, :],
                                        Op.is_gt)
                nc.vector.tensor_sub(fr2[:, :], fr2[:, :], fg[:, :])  # floor
                nc.vector.tensor_scalar(fr2[:, :], fr2[:, :],
                                        -float(NB * S), None, op0=Op.mult)
                nc.vector.tensor_add(iota_isf[:, :], iota_isf[:, :],
                                     fr2[:, :])
            zsel = pool.tile([NODES, 1], f32, tag="zsel")
            nc.vector.tensor_scalar(zsel[:, :], n_keep, 0.0, None,
                                    op0=Op.is_equal)
            p3 = pool.tile([NODES, 8], f32, tag="p3")
            nc.vector.memset(p3[:, :], float(ZCOL))
            for qi, qv in enumerate(QUANTILES):
                zq = pool.tile([NODES, 1], f32, tag="zq")
                nc.vector.tensor_scalar(zq[:, :], n_keep, float(qv), None,
                                        op0=Op.mult)
                rq = pool.tile([NODES, 1], f32, tag="rq")
                nc.vector.tensor_scalar(rq[:, :], zq[:, :], MAGIC, -MAGIC,
                                        op0=Op.add, op1=Op.add)
                g1 = pool.tile([NODES, 1], f32, tag="g1")
                nc.vector.tensor_tensor(g1[:, :], rq[:, :], zq[:, :], Op.is_gt)
                nc.vector.tensor_sub(rq[:, :], rq[:, :], g1[:, :])  # floor
                nc.vector.tensor_tensor(g1[:, :], zq[:, :], rq[:, :], Op.is_gt)
                nc.vector.tensor_add(rq[:, :], rq[:, :], g1[:, :])  # ceil
                nc.vector.tensor_scalar(rq[:, :], rq[:, :], -1.0, 0.0,
                                        op0=Op.add, op1=Op.max)  # k_q
                # pos = i*S + k_q ; if n==0 -> ZCOL
                nc.vector.tensor_add(rq[:, :], rq[:, :], iota_isf[:, :])
                d1 = pool.tile([NODES, 1], f32, tag="d1")
                nc.vector.tensor_scalar(d1[:, :], rq[:, :], -1.0, float(ZCOL),
                                        op0=Op.mult, op1=Op.add)
                nc.vector.tensor_mul(d1[:, :], d1[:, :], zsel[:, :])
                nc.vector.tensor_add(p3[:, qi:qi + 1], rq[:, :], d1[:, :])

            # ---------------- J extraction: kept j indices per node
            iota_j = pool.tile([NODES, N], i32, tag="iota_j")
            nc.gpsimd.iota(iota_j[:, :], pattern=[[-1, N]], base=IOTA_BASE,
                           channel_multiplier=0)
            key = pool.tile([NODES, N], f32, tag="key")
            nc.vector.tensor_copy(key[:, :], iota_j[:, :])
            nc.vector.tensor_mul(key[:, :], key[:, :], keep[:, :])
            jt = pool.tile([NODES, S], f32, tag="jt")
            for r in range(R):
                mx = roll.tile([NODES, 8], f32, tag="mx")
                nc.vector.max(out=mx[:, :], in_=key[:, :])
                if r < R - 1:  # the last round's zap feeds nothing
                    nc.vector.match_replace(out=key[:, :],
                                            in_to_replace=mx[:, :],
                                            in_values=key[:, :],
                                            imm_value=0.0)
                # key value encodes j: j = IOTA_BASE - key; exhausted rows
                # give key 0 -> IOTA_BASE, clamped to the SENT_COL sentinel.
                idxf = roll.tile([NODES, 8], f32, tag="idxf")
                nc.vector.tensor_scalar(idxf[:, :], mx[:, :], -1.0,
                                        float(IOTA_BASE), op0=Op.mult,
                                        op1=Op.add)
                nc.vector.tensor_scalar(jt[:, r * 8:(r + 1) * 8], idxf[:, :],
                                        float(SENT_COL), None, op0=Op.min)

            # J + P3 -> int16 -> DRAM -> wrapped idx layout (16-partition wrap,
            # replicated to the 8 gpsimd core groups)
            j16 = pool.tile([NODES, S], i16, tag="j16")
            nc.vector.tensor_copy(j16[:, :], jt[:, :])
            p16 = pool.tile([NODES, 8], i16, tag="p16")
            nc.vector.tensor_copy(p16[:, :], p3[:, :])
            jdram = dpool.tile([NODES, S], i16, tag="jdram")
            pdram = dpool.tile([NODES, 8], i16, tag="pdram")
            nc.sync.dma_start(jdram[:, :], j16[:, :])
            nc.sync.dma_start(pdram[:, :], p16[:, :])

            # wrapped idx list for the pick gather: unwrapped idx t = q*NB + i
            # lives at pdram addr i*8 + q; partition = t%16 = i%16.
            wrap_pick = pool.tile([128, (NODES * 8) // 16], i16, tag="wrap_pick")
            wp_src = bass.AP(pdram[:, :].tensor, 0,
                             [[8, 16], [1, 8], [8 * 16, NODES // 16]])
            for g in range(8):
                nc.sync.dma_start(wrap_pick[g * 16:(g + 1) * 16, :], wp_src)

            out_sb = pool.tile([NODES, D], f32, tag="out_sb")

            # xt_own / W / b loads
            xo = pool.tile([D, NODES], f32, tag="xo")
            nc.sync.dma_start(xo[:, :], xt_own[:, :])
            w_sb = pool.tile([D, D], f32, tag="w_sb")
            nc.sync.dma_start(w_sb[:, :], w_in[:, :])
            b_sb = pool.tile([D, 1], f32, tag="b_sb")
            nc.sync.dma_start(b_sb[:, :], b_in[:, :])
            wh = pool.tile([D, D], f32, tag="wh")
            scale_w = (1.0 - SELF_RATIO) if USE_SELF else 1.0
            nc.vector.tensor_scalar(wh[:, :], w_sb[:, :], float(scale_w), None,
                                    op0=Op.mult)
            pwt = psum.tile([D, D], f32, tag="ps")
            nc.tensor.transpose(pwt[:, :], wh[:, :], ident[:, :])
            whT = pool.tile([D, D], f32, tag="whT")  # [d, d'] = scale*W^T
            nc.scalar.copy(whT[:, :], pwt[:, :])

            s_mix = pool.tile([D, NODES], f32, tag="s_mix")

            for ib in range(NBATCH):
                nrows = slice(ib * NB, (ib + 1) * NB)
                # unwrapped idx t = i_local*S + s at jdram addr
                # (ib*NB + i_local)*S + s; partition = t%16 = s%16 (S%16==0).
                wrap_j = batchp.tile([128, (NB * S) // 16], i16, tag="wrap_j")
                wj_src = bass.AP(jdram[:, :].tensor, ib * NB * S,
                                 [[1, 16], [S, NB], [16, S // 16]])
                for g in range(8):
                    nc.sync.dma_start(wrap_j[g * 16:(g + 1) * 16, :], wj_src)

                slab = batchp.tile([D, SLABW + 8], f32, tag="slab")
                nc.vector.memset(slab[:, SLABW:], 0.0)
                nc.gpsimd.ap_gather(
                    out_ap=slab[:, :SLABW], in_ap=xt[:, :N + 1],
                    idxs_ap=wrap_j[:, :], channels=128, num_elems=N + 1, d=1,
                    num_idxs=NB * S)

                # ---- Batcher odd-even merge sort of each S-segment
                import bass_rust
                APc = bass.AP
                sl_t = slab[:, :].tensor
                part = list(slab[:, :].ap[0])
                scr = pool.tile([D, max_fam_pairs * NB], f32, tag="scr")
                scr_t = scr[:, :].tensor
                scr_part = list(scr[:, :].ap[0])
                for dist, fams in net:
                    for (s0, step, oc, rl) in fams:
                        dims = [part, [S, NB], [step, oc], [1, rl]]
                        lo = APc(sl_t, s0, [list(d) for d in dims])
                        hi = APc(sl_t, s0 + dist, [list(d) for d in dims])
                        tv = APc(scr_t, 0,
                                 [list(scr_part), [oc * rl, NB], [rl, oc],
                                  [1, rl]])
                        nc.scalar.copy(tv, lo)
                        nc.vector.tensor_tensor(lo, lo, hi, Op.min)
                        nc.vector.tensor_tensor(hi, tv, hi, Op.max)

                # ---- pick quantile slots
                picked = batchp.tile([D, NB * 8], f32, tag="picked")
                if NBATCH == 1:
                    wp = wrap_pick[:, :]
                else:
                    # unwrapped idx t = q*NB + i_local at pdram addr
                    # (ib*NB + i_local)*8 + q; partition = i_local%16.
                    wp_b = bass.AP(pdram[:, :].tensor, ib * NB * 8,
                                   [[8, 16], [1, 8], [8 * 16, NB // 16]])
                    wpt = batchp.tile([128, (NB * 8) // 16], i16, tag="wrap_pb")
                    for g in range(8):
                        nc.sync.dma_start(wpt[g * 16:(g + 1) * 16, :], wp_b)
                    wp = wpt[:, :]
                nc.gpsimd.ap_gather(
                    out_ap=picked[:, :], in_ap=slab[:, :SLABW + 8],
                    idxs_ap=wp, channels=128, num_elems=SLABW + 8, d=1,
                    num_idxs=NB * 8)

                agg = batchp.tile([D, NB], f32, tag="agg")
                nc.vector.tensor_add(agg[:, :], picked[:, 0:NB],
                                     picked[:, NB:2 * NB])
                nc.vector.tensor_add(agg[:, :], agg[:, :],
                                     picked[:, 2 * NB:3 * NB])
                wq = 1.0 / len(QUANTILES)
                if USE_SELF:
                    # s_mix = wq*agg + x ; out = (1-r)*s_mix @ W^T (+b +x)
                    nc.vector.scalar_tensor_tensor(
                        s_mix[:, nrows], agg[:, :], float(wq), xo[:, nrows],
                        op0=Op.mult, op1=Op.add)
                else:
                    nc.vector.tensor_scalar(s_mix[:, nrows], agg[:, :],
                                            float(wq), None, op0=Op.mult)

            # ---------------- output matmul: psum[d',i] = (s*W)[d',d]@s_mix
            pmm = psum.tile([D, NODES], f32, tag="ps")
            nc.tensor.matmul(pmm[:, :], whT[:, :], s_mix[:, :],
                             start=True, stop=True)
            out_di = pool.tile([D, NODES], f32, tag="out_di")
            if RESIDUAL:
                nc.vector.scalar_tensor_tensor(out_di[:, :], pmm[:, :],
                                               b_sb[:, :], xo[:, :],
                                               op0=Op.add, op1=Op.add)
            else:
                nc.vector.tensor_scalar(out_di[:, :], pmm[:, :], b_sb[:, :],
                                        None, op0=Op.add)
            pot = psum.tile([NODES, D], f32, tag="ps")
            nc.tensor.transpose(pot[:, :], out_di[:, :], ident[:, :])
            nc.scalar.copy(out_sb[:, :], pot[:, :])
            nc.sync.dma_start(out_t[:, :], out_sb[:, :])

    nc.finalize()
    return nc


def _get_program(N, D, S):
    key = (N, D, S)
    if key not in _PROGRAMS:
        _PROGRAMS[key] = _build_program(N, D, S)
    return _PROGRAMS[key]


# ---------------------------------------------------------------- host API
def kernel(x, A, W, b):
    try:
        import jax
        if jax.default_backend() != "axon" and any(
            p == "axon" for p in jax._src.xla_bridge.backends()
        ):
            jax.config.update("jax_platforms", "axon")
    except Exception:
        pass
    from concourse.bass_utils import run_bass_kernel_spmd

    x = np.ascontiguousarray(np.asarray(x, dtype=np.float32))
    A = np.ascontiguousarray(np.asarray(A, dtype=np.int32))
    W = np.ascontiguousarray(np.asarray(W, dtype=np.float32))
    b = np.ascontiguousarray(np.asarray(b, dtype=np.float32))
    N, D = x.shape
    NODES = N // N_CORES

    # S (slots per node) is a compile-time shape knob picked from the max
    # degree; used only for dispatch, never folded into the result.
    maxdeg = int((A != 0).sum(axis=1).max())
    S = max(16, int(np.ceil((maxdeg + 1) / 8.0)) * 8)
    S = min(S, N)
    nc = _get_program(N, D, S)

    in_maps = []
    for c in range(N_CORES):
        rows = slice(c * NODES, (c + 1) * NODES)
        in_maps.append({
            "a_slab": A[rows, :],
            "x_full": x,
            "xt_own": np.ascontiguousarray(x[rows, :].T),
            "w_in": W,
            "b_in": b.reshape(D, 1),
        })
    res = run_bass_kernel_spmd(nc, in_maps, core_ids=list(range(N_CORES)))
    out = np.concatenate([res.results[c]["out"] for c in range(N_CORES)],
                         axis=0)
    return out
